# revision 33
# baseline (speedup 1.0000x reference)
"""Causal multi-head attention (B=4, S=2048, D=1024, H=16, RoPE) on 8 trn2 cores.

Sharding: core c -> batch c//2, head-half c%2 (8 heads / 512 dims per core).
Each core computes QKV projections for its head slice, RoPE, causal flash
attention, and a partial output projection with its Wo column slice; the host
sums the two partials per batch (the tensor-parallel all-reduce) and
transposes back.

v3 (vs the 253us v2): the PV matmul is FLIPPED — the exp'd probability tile
is the stationary operand and [V] the moving one, so each matmul streams only
64 output columns (charged N=64) instead of 512 at M=65.  Output lands as
[q, dv] in PSUM, one 128-q-sub-block per accumulation chain (the chain for
q-block g closes at its diagonal kt=g, letting early blocks finish early).
  - softmax row-sums via N=1 ones-column matmuls into a shared 1-bank psum
    tile; the normalizer is then a per-PARTITION scalar in the [q, dv]
    layout, so normalization is one DVE reciprocal + a broadcast-AP multiply
    (no DRAM round-trip, no PE broadcast matmuls, no deferred-scale pass)
  - the scaled [q, dv] tile returns to [dv, q] via the DMA-engine XBAR
    transpose (zero PE/DVE cost); HOP's q-blocks are padded to stride 132 so
    the 3D out-AP can't be flattened (which would scramble the block order)
  - PSUM: S psum 2x[128,2,512] (4 banks) + outq 2x[128,4,128] (2) + norm
    rowsums (1) + one phase-C bank = exactly 8; phase-C column chains are
    spread one-per-unit so the single-bank ring never stalls PE, with the
    tail burst drawing 2-chain tiles from the freed S pool instead
"""

import numpy as np

import concourse.bass as bass
import concourse.bacc as bacc
import concourse.mybir as mybir
import concourse.tile as tile
from concourse.bass import ds, ts
from concourse.bass_utils import run_bass_kernel_spmd

F32 = mybir.dt.float32
BF16 = mybir.dt.bfloat16

B, S, D, H, DK = 4, 2048, 1024, 16, 64
THETA = 10000.0
NH = 8  # heads per core
HD = NH * DK  # 512 head dims per core
P = 128
EXPF = mybir.ActivationFunctionType.Exp


DEFAULT_TUNE = dict(
    c_slack=4,     # units between transpose and first eligible C pop
    c_space=2,     # min units between consecutive C chain launches
    outq_bufs=1,
    lag=2,         # software-pipeline lag (units) between S/exp and PV
    nwarm=20,      # attention units pre-issued during the V pass
)


def build_attention_nc(nrep=1, tune=None):
    t = dict(DEFAULT_TUNE)
    t.update(tune or {})
    nc = bacc.Bacc("TRN2", target_bir_lowering=False, debug=False)

    xT = nc.dram_tensor("xT", [D, S], BF16, kind="ExternalInput")
    wqT = nc.dram_tensor("wqT", [D, HD], BF16, kind="ExternalInput")
    wkT = nc.dram_tensor("wkT", [D, HD], BF16, kind="ExternalInput")
    wvT = nc.dram_tensor("wvT", [D, HD], BF16, kind="ExternalInput")
    woT = nc.dram_tensor("woT", [HD, D], BF16, kind="ExternalInput")
    cosP = nc.dram_tensor("cosP", [P, S], BF16, kind="ExternalInput")
    sinP = nc.dram_tensor("sinP", [P, S], BF16, kind="ExternalInput")
    trimask = nc.dram_tensor("trimask", [P, 2 * P], BF16, kind="ExternalInput")
    onesc = nc.dram_tensor("onesc", [P, P], BF16, kind="ExternalInput")
    outT = nc.dram_tensor("outT", [D, S], BF16, kind="ExternalOutput")

    with tile.TileContext(nc) as tc:
        if nrep == 1:
            _attention_tile(
                tc, xT, wqT, wkT, wvT, woT, cosP, sinP, trimask, onesc, outT, t
            )
        else:
            with tc.For_i(0, nrep, 1):
                _attention_tile(
                    tc, xT, wqT, wkT, wvT, woT, cosP, sinP, trimask, onesc, outT, t
                )
    nc.compile()
    return nc


def _attention_tile(tc, xT, wqT, wkT, wvT, woT, cosP, sinP, trimask, onesc, outT, t=None):
    t = t or dict(DEFAULT_TUNE)
    nc = tc.nc

    with (
        tc.tile_pool(name="qkv", bufs=1) as qkv,
        tc.tile_pool(name="ptile", bufs=t["lag"] + 4) as ptile,
        tc.tile_pool(name="swpool", bufs=2) as swpool,
        tc.tile_pool(name="rcp", bufs=2) as rcpp,
        tc.tile_pool(name="hbuf", bufs=2) as hbufp,
        tc.tile_pool(name="obpool", bufs=2) as obpool,
        tc.tile_pool(name="psA", bufs=2, space="PSUM") as psA,
        tc.tile_pool(name="psS", bufs=2, space="PSUM") as psS,
        tc.tile_pool(name="outqp", bufs=t["outq_bufs"], space="PSUM") as outqp,
        tc.tile_pool(name="normp", bufs=1, space="PSUM") as normp,
    ):
        # ---- persistent tiles ----
        x_sb = qkv.tile([P, 8, S], BF16, tag="x")     # [k%128, k//128, t]
        QT = qkv.tile([P, 4, S], BF16, tag="QT")      # [d'%128, d'//128, t]
        KT = qkv.tile([P, 4, S], BF16, tag="KT")
        VP = qkv.tile([P, 16, HD], BF16, tag="VP")    # [t%128, t//128, dv]
        # HOP: [dv%128, m, qblk, q] with q-blocks padded to stride 144 (the
        # XBAR transpose rounds the block stride down to a multiple of 16)
        HOP = qkv.tile([P, 4, 16, 144], BF16, tag="HOP")
        cos_sb = qkv.tile([P, S], BF16, tag="cos")
        sin_sb = qkv.tile([P, S], BF16, tag="sin")
        wq_sb = qkv.tile([P, 8, HD], BF16, tag="wq")
        wk_sb = qkv.tile([P, 8, HD], BF16, tag="wk")
        wv_sb = qkv.tile([P, 8, HD], BF16, tag="wv")
        wo_sb = qkv.tile([P, 4, D], BF16, tag="wo")
        tri_sb = qkv.tile([P, 2, P], BF16, tag="tri")
        ones_col = qkv.tile([P, 1], BF16, tag="ones1")

        # ---- bulk input DMAs, all up front, in first-use order ----
        xT_t = xT.ap().rearrange("(o p) t -> p o t", p=P)
        wq_src = wqT.ap().rearrange("(o p) d -> p o d", p=P)
        nc.sync.dma_start(wq_sb[:, 0:2, :], wq_src[:, 0:2, :])
        nc.scalar.dma_start(x_sb[:, 0:2, ds(0, 512)], xT_t[:, 0:2, ds(0, 512)])
        nc.sync.dma_start(wq_sb[:, 2:4, :], wq_src[:, 2:4, :])
        nc.scalar.dma_start(x_sb[:, 2:4, ds(0, 512)], xT_t[:, 2:4, ds(0, 512)])
        nc.sync.dma_start(wq_sb[:, 4:8, :], wq_src[:, 4:8, :])
        nc.scalar.dma_start(x_sb[:, 4:8, ds(0, 512)], xT_t[:, 4:8, ds(0, 512)])
        nc.scalar.dma_start(wk_sb, wkT.ap().rearrange("(o p) d -> p o d", p=P))
        for tci in range(1, 4):
            nc.sync.dma_start(x_sb[:, :, ds(tci * 512, 512)], xT_t[:, :, ds(tci * 512, 512)])
        nc.sync.dma_start(cos_sb, cosP.ap())
        nc.sync.dma_start(sin_sb, sinP.ap())
        nc.sync.dma_start(tri_sb, trimask.ap().rearrange("p (e q) -> p e q", e=2))
        nc.sync.dma_start(ones_col, onesc.ap()[:, 0:1])
        nc.scalar.dma_start(wv_sb, wvT.ap().rearrange("(o p) d -> p o d", p=P))
        nc.sync.dma_start(wo_sb, woT.ap().rearrange("(m p) o -> p m o", p=P))

        normt = normp.tile([P, 8, 1], F32, tag="norm")
        # rewritten (same value) after each block's final recips so the next
        # block's bank-clearing first norm matmul cannot pass them
        ones_norm = rcpp.tile([P, 1], BF16, tag="onesn", bufs=1)
        nc.vector.tensor_copy(ones_norm, ones_col)
        outT_t = outT.ap().rearrange("(o p) t -> p o t", p=P)

        # ---------------- emission helpers ----------------

        def qk_chain(tci, dst, w_sb, j):
            ps = psA.tile([P, 512], F32, tag="psA", name=f"qk{tci}{j}")
            for i in range(8):
                nc.tensor.matmul(
                    ps,
                    lhsT=w_sb[:, i, ts(j, P)],
                    rhs=x_sb[:, i, ds(tci * 512, 512)],
                    start=(i == 0),
                    stop=(i == 7),
                )
            nc.scalar.copy(dst[:, j, ds(tci * 512, 512)], ps)
            if tci % 2 == 1:
                # RoPE for this (tensor, j) half-row: pair-swap via 4
                # partition-block DMAs + 3 DVE bf16 TTs
                hsl = ds((tci // 2) * 1024, 1024)
                qsw = swpool.tile([P, 1024], BF16, tag="qsw")
                for blk in range(4):
                    sb = blk + (1 if blk % 2 == 0 else -1)
                    nc.sync.dma_start(
                        qsw[blk * 32 : blk * 32 + 32, :],
                        dst[sb * 32 : sb * 32 + 32, j, hsl],
                    )
                tmp = swpool.tile([P, 1024], BF16, tag="rtmp")
                nc.vector.tensor_mul(tmp, cos_sb[:, hsl], dst[:, j, hsl])
                nc.vector.tensor_mul(qsw, sin_sb[:, hsl], qsw)
                nc.vector.tensor_add(dst[:, j, hsl], tmp, qsw)

        def v_chain(kt):
            ps = psA.tile([P, 512], F32, tag="psA", name=f"v{kt}")
            tci, tt = kt // 4, kt % 4
            for i in range(8):
                nc.tensor.matmul(
                    ps,
                    lhsT=x_sb[:, i, ds(tci * 512 + tt * P, P)],
                    rhs=wv_sb[:, i, :],
                    start=(i == 0),
                    stop=(i == 7),
                )
            nc.vector.tensor_copy(VP[:, kt, :], ps)

        blk_q = {}    # (qb, m) -> outq psum tile
        blk_hb = {}   # (qb, m) -> scaled [q, dv] staging tile
        c_jobs = []   # pending phase-C column chains (cqb, ot, due_unit)
        ob_state = {}
        unit_no = [0]
        next_c_ok = [0]

        def emit_c_job(pool_tile=None, evac_eng=None, force=False):
            if not c_jobs:
                return
            if not force and (c_jobs[0][2] > unit_no[0]
                              or next_c_ok[0] > unit_no[0]):
                return
            cqb, ot, _ = c_jobs.pop(0)
            next_c_ok[0] = unit_no[0] + t["c_space"]
            key = (cqb, ot // 2)
            if key not in ob_state:
                ob_state[key] = obpool.tile(
                    [P, 2, 512], BF16, tag="ob", name=f"ob{cqb}_{ot // 2}"
                )
            ob2 = ob_state[key]
            k = ot % 2
            if pool_tile is None:
                ps = psA.tile([P, 512], F32, tag="psA", name=f"c{cqb}_{ot}")
            else:
                ps = pool_tile
            for mm in range(4):
                nc.tensor.matmul(
                    ps,
                    lhsT=wo_sb[:, mm, ts(ot, P)],
                    rhs=HOP[:, mm, 4 * cqb : 4 * cqb + 4, 0:P],
                    start=(mm == 0),
                    stop=(mm == 3),
                )
            eng = evac_eng or nc.vector
            if eng is nc.scalar:
                eng.copy(ob2[:, k, :], ps)
            else:
                eng.tensor_copy(ob2[:, k, :], ps)
            if k == 1:
                del ob_state[key]
                nc.gpsimd.dma_start(
                    outT_t[:, ot - 1 : ot + 1, ds(cqb * 512, 512)], ob2
                )

        def s_exp_unit(qb, m, kt):
            roff = kt - 4 * qb
            c0 = max(0, 128 * roff)
            s2 = psS.tile([P, 2, 512], F32, tag="s")
            for e in range(2):
                rb = e * 64
                nc.tensor.matmul(
                    s2[:, e, c0:],
                    lhsT=KT[rb : rb + 64, m, ts(kt, P)],
                    rhs=QT[rb : rb + 64, m, ds(qb * 512 + c0, 512 - c0)],
                    start=True,
                    stop=True,
                )
            pt2 = ptile.tile([P, 2, 512], BF16, tag="pt")
            nc.scalar.activation(pt2[:, :, c0:], s2[:, :, c0:], EXPF, scale=0.125)
            if roff >= 0:
                nc.vector.tensor_mul(
                    pt2[:, :, ds(c0, P)], pt2[:, :, ds(c0, P)], tri_sb
                )
            return pt2, c0

        def finish_g(qb, m, g):
            """Normalize + transpose one just-closed 128-query sub-block."""
            hb = blk_hb[(qb, m)]
            oq = blk_q[(qb, m)]
            for e in range(2):
                rcg = rcpp.tile([P, 1], F32, tag="rcg", bufs=8, name=f"rcg{g}{e}")
                nc.vector.reciprocal(rcg, normt[:, e * 4 + g, 0:1])
                nc.vector.tensor_scalar_mul(
                    hb[:, g, ds(e * 64, 64)], oq[:, g, ds(e * 64, 64)], rcg
                )
            nc.sync.dma_start(
                HOP[:, m, 4 * qb + g, 0:P], hb[:, g, :], transpose=True
            )
            if g == 3:
                nc.vector.tensor_copy(ones_norm, ones_col)
                blk_q.pop((qb, m))
                blk_hb.pop((qb, m))
                if qb >= 1:
                    c_jobs.append((qb - 1, 2 * m, unit_no[0] + t["c_slack"]))
                    c_jobs.append((qb - 1, 2 * m + 1, unit_no[0] + t["c_slack"]))

        def pv_unit(qb, m, kt, pt2, c0):
            roff = kt - 4 * qb
            if kt == 0:
                blk_q[(qb, m)] = outqp.tile(
                    [P, 4, P], F32, tag="oq", name=f"oq{qb}{m}"
                )
                blk_hb[(qb, m)] = hbufp.tile(
                    [P, 4, P], BF16, tag="hb", name=f"hb{qb}{m}"
                )
            oq = blk_q[(qb, m)]
            # start=True clears has_written for the ENTIRE psum bank, so only
            # the first matmul into each bank this block carries it; every
            # other chain's first write lands on cleared bits and overwrites.
            for e in range(2):
                vsl = m * 128 + e * 64
                for g in range(max(0, roff), 4):
                    lhs = pt2[:, e, ds(g * 128, P)]
                    first = kt == 0 and e == 0 and g == 0
                    nc.tensor.matmul(
                        oq[:, g, ds(e * 64, 64)],
                        lhsT=lhs,
                        rhs=VP[:, kt, ds(vsl, 64)],
                        start=first,
                        stop=(kt == 4 * qb + g),
                        skip_group_check=True,
                    )
                    nc.tensor.matmul(
                        normt[:, e * 4 + g, :],
                        lhsT=lhs,
                        rhs=ones_norm,
                        start=first,
                        stop=(kt == 4 * qb + g),
                        skip_group_check=True,
                    )
            if roff >= 0:
                finish_g(qb, m, roff)

        pipe = []
        LAG = t["lag"]
        v_next = [0]

        def v_upto(kt):
            # guard: the PV consuming VP[kt] must come after v_chain(kt) in
            # program order or the dep tracker can't order the write first
            while v_next[0] <= kt:
                v_chain(v_next[0])
                v_next[0] += 1

        def drain_one():
            q_, m_, k_, p_, c_ = pipe.pop(0)
            unit_no[0] += 1
            v_upto(k_)
            pv_unit(q_, m_, k_, p_, c_)
            emit_c_job()

        def emit_unit(qb, m, kt):
            if len(pipe) >= LAG:
                drain_one()
            pt2, c0 = s_exp_unit(qb, m, kt)
            pipe.append((qb, m, kt, pt2, c0))

        # ---------------- emission schedule ----------------
        # segment 1: QK for the first token half (+ RoPE at tci=1)
        for tci in (0, 1):
            for dst, w_sb in ((QT, wq_sb), (KT, wk_sb)):
                for j in range(4):
                    qk_chain(tci, dst, w_sb, j)

        # segment 2: V kt0-7 + remaining QK, interleaved with qb0/qb1 units
        v_upto(3)
        seg2_proj = (
            [("qk", 2, QT, wq_sb, j) for j in range(4)]
            + [("qk", 2, KT, wk_sb, j) for j in range(4)]
            + [("v", kt) for kt in (4, 5)]
            + [("qk", 3, QT, wq_sb, j) for j in range(4)]
            + [("v", kt) for kt in (6, 7)]
            + [("qk", 3, KT, wk_sb, j) for j in range(4)]
        )
        seg2_units = [
            (qb, m, kt) for qb in (0, 1) for m in range(4)
            for kt in range(4 * qb + 4)
        ]
        ui = 0
        for pi, p in enumerate(seg2_proj):
            if p[0] == "qk":
                qk_chain(*p[1:])
            else:
                v_upto(p[1])
            due = len(seg2_units) * (pi + 1) // len(seg2_proj)
            while ui < due:
                emit_unit(*seg2_units[ui])
                ui += 1
        while ui < len(seg2_units):
            emit_unit(*seg2_units[ui])
            ui += 1

        # segment 3: V kt8-15 + qb2, then qb3
        v_upto(11)
        rest_v = [12, 13, 14, 15]
        seg3_units = [(2, m, kt) for m in range(4) for kt in range(12)]
        for i, u in enumerate(seg3_units):
            emit_unit(*u)
            if i % 3 == 2 and rest_v:
                v_upto(rest_v.pop(0))
        for u in [(3, m, kt) for m in range(4) for kt in range(16)]:
            emit_unit(*u)
        while pipe:
            drain_one()

        # tail: qb3 column chains across the freed S/psA banks
        for ot in range(8):
            c_jobs.append((3, ot, 0))
        ti = 0
        while c_jobs:
            if ti % 3 == 2:
                aT = psA.tile([P, 512], F32, tag="psA", name=f"ctl{ti}")
                emit_c_job(
                    pool_tile=aT,
                    evac_eng=(nc.scalar if ti % 2 else nc.vector),
                    force=True,
                )
                ti += 1
                continue
            sT = psS.tile([P, 2, 512], F32, tag="s", name=f"ctail{ti}")
            for k in range(2):
                if not c_jobs:
                    break
                emit_c_job(
                    pool_tile=sT[:, k, :],
                    evac_eng=(nc.scalar if (ti + k) % 2 else nc.vector),
                    force=True,
                )
            ti += 2


# ---------------- host side ----------------

def _host_tables():
    import ml_dtypes

    i = np.arange(32, dtype=np.float32)
    inv_freq = (THETA ** (2.0 * i / DK)).astype(np.float32)
    t = np.arange(S, dtype=np.float32)
    ang = t[:, None] / inv_freq[None, :]  # [S, 32]
    c = np.cos(ang).astype(np.float32).T  # [32, S]
    sn = np.sin(ang).astype(np.float32).T
    cosP = np.tile(c, (4, 1))  # [128, S]
    sinP = np.tile(sn, (4, 1))
    sign = np.repeat(np.array([-1.0, 1.0, -1.0, 1.0], dtype=np.float32), 32)
    sinP = sinP * sign[:, None]

    kk = np.arange(P)[:, None]
    qq = np.arange(P)[None, :]
    keep = (kk <= qq).astype(ml_dtypes.bfloat16)  # [128,128]
    trimask = np.tile(keep, (1, 2))  # [128, 2*128] (both head halves)
    bf = ml_dtypes.bfloat16
    return cosP.astype(bf), sinP.astype(bf), trimask


_PERM = np.concatenate(
    [np.concatenate([h * 64 + np.arange(0, 64, 2), h * 64 + np.arange(1, 64, 2)])
     for h in range(NH)]
)

_NC_CACHE = {}


def make_in_maps(x, Wq, Wk, Wv, Wo):
    import ml_dtypes

    bf = ml_dtypes.bfloat16
    cosP, sinP, trimask = _host_tables()
    in_maps = []
    for c in range(8):
        b, hh = c // 2, c % 2
        sl = slice(hh * HD, (hh + 1) * HD)
        in_maps.append(
            {
                "xT": np.ascontiguousarray(x[b].T).astype(bf),
                "wqT": np.ascontiguousarray(Wq[sl, :][_PERM].T).astype(bf),
                "wkT": np.ascontiguousarray(Wk[sl, :][_PERM].T).astype(bf),
                "wvT": np.ascontiguousarray(Wv[sl, :].T).astype(bf),
                "woT": np.ascontiguousarray(Wo[:, sl].T).astype(bf),
                "cosP": cosP,
                "sinP": sinP,
                "trimask": trimask,
                "onesc": np.ones((P, P), dtype=bf),
            }
        )
    return in_maps


def gather_out(core_outs):
    out = np.empty((B, S, D), dtype=np.float32)
    for b in range(B):
        a = np.asarray(core_outs[2 * b]["outT"], dtype=np.float32)
        bb = np.asarray(core_outs[2 * b + 1]["outT"], dtype=np.float32)
        out[b] = (a + bb).T
    return out


def kernel(x, Wq, Wk, Wv, Wo):
    x = np.asarray(x, dtype=np.float32)
    Wq = np.asarray(Wq, dtype=np.float32)
    Wk = np.asarray(Wk, dtype=np.float32)
    Wv = np.asarray(Wv, dtype=np.float32)
    Wo = np.asarray(Wo, dtype=np.float32)

    if "nc" not in _NC_CACHE:
        _NC_CACHE["nc"] = build_attention_nc()
    nc = _NC_CACHE["nc"]

    in_maps = make_in_maps(x, Wq, Wk, Wv, Wo)
    res = run_bass_kernel_spmd(nc, in_maps, core_ids=list(range(8)))
    return gather_out(res.results)
